# revision 11
# baseline (speedup 1.0000x reference)
"""Trainium2 Bass kernel for nn_ADDSLoss (retrieval_knn).

Math:
  pred = points @ R_p^T + t_p ; gt = points @ R_g^T + t_g          (per batch)
  out  = mean_{b,m} min_n ||pred[b,m] - gt[b,n]||

Device formulation (per batch):
  d2[m,n] = pn[m] + gn[n] - 2*pred[m].gt[n] = sum_k L[k,m]*R[k,n]   (K=5)
  with L = W_L @ F, R = W_R @ F, where F is the degree-2 polynomial
  feature map of the raw model points, built on host:
    F rows = [x, y, z, 1, x^2, y^2, z^2, x*y, y*z, z*x]   [10, M]
  and W_L/W_R are tiny per-batch 5x10 matrices (host, O(B) work).
  min_n sqrt(max(d2,0)) == sqrt(max(min_n d2, 0)), so sqrt happens on
  M values per batch, not M^2.

Sharding: data-parallel over B across the 8 cores (2 batches/core). Each
core returns its partial sum of min-distances; the final mean is the sum
of the 8 partials / (B*M) on host (a trivial 8-float reduction).

Device pipeline per core:
  phase 0: DMA F (host-built features) + stacked W matrix.
  phase 1: transform matmuls (fp32r, K=10, all four 5x10 W's stacked in
           one [10,128] lhsT) -> PSUM -> cast-evac to fp16 -> DMA the
           four 5-row side blocks to partition offsets {0,32,64,96}.
  phase 2 (DRAIN="cdve"): per (m-tile, batch, psum-tile): 4 fp16 K=5
           matmuls row-packed via tile_position -> [128,2048] d2 in
           PSUM; ACT copies cols [1024:2048] to SBUF fp16; one custom
           DVE ucode op (MIN_MIN_REDUCE_ANT: out = min(in0,in1),
           accum_out = min(s0, min out)) mins the copied half against
           the PSUM half AND min-reduces in a single DVE pass.
  phase 2 (DRAIN="tree"): baseline-style split drain - DVE
           tensor_reduce(min) eats XDVE cols from PSUM; ACT evacuates
           the rest to fp16 SBUF where a 3-level tensor_tensor(min)
           tree runs in DVE 2x mode (software-pipelined across units).
  phase 3: relu -> sqrt -> free-axis sum -> partition all-reduce -> DMA.
"""

import functools
import os

import numpy as np

B, M = 16, 4096
NCORES = 8
BPC = B // NCORES        # batches per core
NCH = M // 512           # 8 rhs chunks of 512
NMT = M // 128           # 32 m-tiles of 128
NSIDES = 2 * BPC         # L0, R0, L1, R1
BIG = 3.0e38             # running-min init (finite to avoid inf handling)
DRAIN = os.environ.get("ADDS_DRAIN", "cdve")  # "cdve" | "tree"


def _register_custom_dve():
    """Register MIN_MIN_REDUCE_ANT: out=min(in0,in1); accum=min(s0,min out)."""
    import concourse.dve_ops as dve_ops
    from concourse.dve_spec import Spec, Src0, Src1, C0, minn, lower, _has_src1
    from concourse.dve_uop import DveOpSpec

    name = "MIN_MIN_REDUCE_ANT"
    for o in dve_ops.OPS:
        if o.name == name:
            return o

    def _ref(in0, in1, c0, c1, c2):
        b = np.minimum(in0.astype(np.float32), in1.astype(np.float32)).astype(
            np.float32
        )
        return b, np.minimum(
            c0, b.reshape(b.shape[0], -1).min(axis=-1, keepdims=True)
        )

    spec = Spec(body=minn(Src0, Src1), accum=minn, accum_init=C0, reference=_ref)
    shas = {
        ver: DveOpSpec(
            name=name, opcode=0, uops=lower(spec, ver=ver), rd1_en=_has_src1(spec)
        ).sha(ver)
        for ver in ("v3", "v4")
    }
    op = dve_ops.DveOp(name, spec, subdim=False, uops_sha=shas)
    row = dve_ops._CUSTOM_DVE_ROW_BASE + len(dve_ops.OPS)
    assert row < 0x20
    dve_ops.OPS.append(op)
    dve_ops._SUB_OPCODE_FOR_NAME[name] = row
    dve_ops.CUSTOM_DVE_SPECS[name] = spec
    return op


def _quad_row(Rm, t):
    """Feature-space weights of ||R p + t||^2 over
    [x, y, z, 1, x2, y2, z2, xy, yz, zx]."""
    A = Rm.T @ Rm
    row = np.zeros(10, np.float64)
    row[0:3] = 2.0 * (Rm.T @ t)
    row[3] = float(t @ t)
    row[4:7] = np.diag(A)
    row[7] = 2.0 * A[0, 1]
    row[8] = 2.0 * A[1, 2]
    row[9] = 2.0 * A[0, 2]
    return row


def _w_pred(Rp, tp):
    W = np.zeros((5, 10), np.float64)
    W[0:3, 0:3] = Rp
    W[0:3, 3] = tp
    W[3, 3] = 1.0
    W[4] = _quad_row(Rp, tp)
    return W


def _w_gt(Rg, tg):
    W = np.zeros((5, 10), np.float64)
    W[0:3, 0:3] = -2.0 * Rg
    W[0:3, 3] = -2.0 * tg
    W[3] = _quad_row(Rg, tg)
    W[4, 3] = 1.0
    return W


@functools.lru_cache(maxsize=1)
def _build_graph():
    import concourse.mybir as mybir
    import concourse.tile as tile
    from concourse import bacc

    f32 = mybir.dt.float32
    f16 = mybir.dt.float16  # 16-bit operand dtype for the main matmuls
    AL = mybir.AluOpType

    myop = _register_custom_dve() if DRAIN == "cdve" else None

    nc = bacc.Bacc(
        "TRN2", target_bir_lowering=False, debug=False, num_devices=NCORES
    )
    f32r = mybir.dt.float32r
    feat_d = nc.declare_dram_parameter("feat", [10, M], f32r, isOutput=False)
    wm_d = nc.declare_dram_parameter("wmat", [10, 128], f32r, isOutput=False)
    out_d = nc.declare_dram_parameter("out", [1, 1], f32, isOutput=True)

    with tile.TileContext(nc) as tc:
        with (
            tc.tile_pool(name="constp", bufs=1) as constp,
        ):
            F = constp.tile([10, M], f32r)
            wm = constp.tile([10, 128], f32r)
            sbig = constp.tile([128, M], f16)
            mind = constp.tile([128, 2 * NMT * BPC], f32)
            mindc = constp.tile([128, NMT * BPC], f32)
            dist = constp.tile([128, NMT * BPC], f32)
            colsum = constp.tile([128, 1], f32)
            dummy = constp.tile([128, 1], f32)
            # per-side operand tensors replicated at partition offsets
            # {0,32,64,96} so four K=5 matmuls can run concurrently in the
            # PE array via tile_position row-packing; split into halves so
            # phase 2 can start as soon as the first half is transformed
            lrh = [
                [
                    constp.tile([128, M // 2], f16, name=f"lr{s}h{h}")
                    for h in range(2)
                ]
                for s in range(NSIDES)
            ]

            # ---- phase 0: inputs ----
            nc.sync.dma_start(wm[:], wm_d[:])
            # touch Sqrt now so its ACT table set loads during the prelude
            # (Copy lives in every set, so no second table switch later);
            # scale=0/bias=1 keeps the probe input in sqrt's valid range
            nc.scalar.activation(
                dummy[0:10, 0:1],
                wm[:, 0:1],
                mybir.ActivationFunctionType.Sqrt,
                bias=1.0,
                scale=0.0,
            )
            # chunked so the first transform matmul starts after 1/8 of F;
            # issue alternately on two queues so the half-0 chunks land fast
            for q in range(NCH):
                eng = nc.sync if q % 2 == 0 else nc.scalar
                eng.dma_start(
                    F[:, q * 512 : (q + 1) * 512],
                    feat_d[:, q * 512 : (q + 1) * 512],
                )

            # ---- phase 1: L/R transforms (fp32 exact) ----
            # All four 5x10 transform matrices are stacked into one
            # [10, 128] lhsT (side s at columns 32s..32s+5): the fp32
            # matmul cost is per-row, independent of M, so one stacked
            # matmul per chunk does the work of four.  The [128, 512]
            # outputs are cast-evacuated to fp16 and the four 5-row
            # blocks DMA'd to partition 0 where the main matmuls expect
            # their K operands.
            with tc.tile_pool(name="pst", bufs=4, space="PSUM") as pst:
                for cc in range(NCH):
                    h, hc = cc // (NCH // 2), cc % (NCH // 2)
                    tp = pst.tile([128, 512], f32, tag="tp")
                    nc.tensor.matmul(tp[:], wm[:], F[:, cc * 512 : (cc + 1) * 512])
                    csl = slice(cc * 512, (cc + 1) * 512)
                    if cc % 2 == 0:
                        nc.scalar.copy(sbig[:, csl], tp[:])
                    else:
                        nc.vector.tensor_copy(sbig[:, csl], tp[:])
                    if hc == NCH // 2 - 1:
                        # replicate this half's four 5-row side blocks to the
                        # four 32-row offsets.  DMA *issue* costs ~800ns per
                        # descriptor on a sequencer, so spread the 16 DMAs
                        # over four otherwise-idle sequencers
                        hof = h * (M // 2)
                        # keep ACT's queue clear of DMA issue cost: its
                        # sequencer must start phase-2 copies immediately
                        emitters = [nc.sync, nc.gpsimd, nc.sync, nc.gpsimd]
                        for s in range(NSIDES):
                            for i in range(4):
                                emitters[i].dma_start(
                                    lrh[s][h][32 * i : 32 * i + 5, :],
                                    sbig[32 * s : 32 * s + 5,
                                         hof : hof + M // 2],
                                )

            # ---- phase 2: pairwise d2 + min-reduce ----
            if DRAIN == "cdve":
                # Per [128,2048] PSUM tile: ACT copies the top half to fp16
                # SBUF; one custom DVE op computes min(copied, psum-bottom)
                # elementwise AND min-reduces it to mind[:, col] in a single
                # pass (2 d2 columns consumed per DVE cycle-column).
                # Quarter-tile schedule: four [128,1024] PSUM slots per
                # iteration.  ACT copies quarters q1/q3 to SBUF while the
                # custom ops pair them with q0/q2 straight from PSUM, so
                # the copy is off the DVE critical path and the copy-slots
                # free early for the next iteration's matmuls.
                # (half, b, mt) order: all half-0 units run before any
                # half-1 unit, so phase 2 starts as soon as the half-0
                # operand replicas land (half-1 transforms overlap it).
                # Per unit: copy-quarter matmuls emitted first (its PSUM
                # slot frees early via the ACT copy), custom-quarter
                # second; row-groups alternate between units so adjacent
                # quarters overlap in the PE array.
                with tc.tile_pool(name="psm", bufs=4, space="PSUM") as psm, \
                     tc.tile_pool(name="cbp", bufs=4) as cbp, \
                     tc.tile_pool(name="dmp", bufs=3) as dmp:
                    u = 0
                    for half in range(2):
                        for b in range(BPC):
                            for mt in range(NMT):
                                lb = lrh[2 * b][mt // (NMT // 2)]
                                rbh = lrh[2 * b + 1][half]
                                mt0 = (mt % (NMT // 2)) * 128
                                mt1 = mt0 + 128
                                pq = [None, None]
                                for qi in range(2):  # 0 = copy-q, 1 = custom-q
                                    c0 = (1 - qi) * 1024
                                    pt = psm.tile([128, 1024], f32, tag="ps")
                                    for c in range(2):
                                        pos = 64 * ((u + qi) % 2) + 32 * c
                                        nc.tensor.matmul(
                                            pt[:, c * 512 : (c + 1) * 512],
                                            lb[pos : pos + 5, mt0:mt1],
                                            rbh[pos : pos + 5,
                                                c0 + c * 512 : c0 + (c + 1) * 512],
                                            tile_position=(pos, 0),
                                        )
                                    pq[qi] = pt
                                cb = cbp.tile([128, 1024], f32, tag="cb")
                                dmpt = dmp.tile([128, 1024], f32, tag="dm")
                                nc.scalar.copy(cb[:], pq[0][:])
                                col = 2 * (mt * BPC + b) + half
                                nc.vector._custom_dve(
                                    myop,
                                    out=dmpt[:],
                                    in0=cb[:],
                                    in1=pq[1][:],
                                    s0=BIG,
                                    s1=0.0,
                                    accum_out=mind[:, col : col + 1],
                                )
                                u += 1
            else:
                # baseline-style split drain, rebalanced: DVE
                # tensor_reduce(min) eats XDVE cols straight from PSUM;
                # ACT evacuates the rest to fp16 SBUF for a 3-level DVE
                # 2x tensor_tensor(min) tree (pipelined across units).
                XDVE = 1000
                E = 4096 - XDVE
                minda = mind  # reuse: cols 0..63 direct, 64..127 tree

                def emit_tree(eb, col):
                    l1 = ebp.tile([128, E // 2], f16, tag="l1")
                    l2 = ebp.tile([128, E // 4], f16, tag="l2")
                    l3 = ebp.tile([128, E // 8], f16, tag="l3")
                    nc.vector.tensor_tensor(
                        l1[:], eb[:, 0 : E // 2], eb[:, E // 2 : E], AL.min
                    )
                    nc.vector.tensor_tensor(
                        l2[:], l1[:, 0 : E // 4], l1[:, E // 4 : E // 2], AL.min
                    )
                    nc.vector.tensor_tensor(
                        l3[:], l2[:, 0 : E // 8], l2[:, E // 8 : E // 4], AL.min
                    )
                    nc.vector.tensor_reduce(
                        minda[:, 64 + col : 65 + col],
                        l3[:],
                        axis=mybir.AxisListType.X,
                        op=AL.min,
                    )

                with tc.tile_pool(name="psm", bufs=2, space="PSUM") as psm, \
                     tc.tile_pool(name="ebp", bufs=4) as ebp:
                    pending = None
                    for b in range(BPC):
                        for mt in range(NMT):
                            lb = lrh[2 * b][mt // (NMT // 2)]
                            rbh0 = lrh[2 * b + 1][0]
                            rbh1 = lrh[2 * b + 1][1]
                            mt0 = (mt % (NMT // 2)) * 128
                            mt1 = mt0 + 128
                            col = mt * BPC + b
                            eb = ebp.tile([128, E], f16, tag="eb")
                            pa = psm.tile([128, 2048], f32, tag="ps")
                            for c in range(4):
                                nc.tensor.matmul(
                                    pa[:, c * 512 : (c + 1) * 512],
                                    lb[32 * c : 32 * c + 5, mt0:mt1],
                                    rbh0[32 * c : 32 * c + 5,
                                         c * 512 : (c + 1) * 512],
                                    tile_position=(32 * c, 0),
                                )
                            nc.vector.tensor_reduce(
                                mind[:, col : col + 1],
                                pa[:, 0:XDVE],
                                axis=mybir.AxisListType.X,
                                op=AL.min,
                            )
                            nc.scalar.copy(eb[:, 0 : 2048 - XDVE], pa[:, XDVE:2048])
                            pb = psm.tile([128, 2048], f32, tag="ps")
                            for c in range(4):
                                nc.tensor.matmul(
                                    pb[:, c * 512 : (c + 1) * 512],
                                    lb[32 * c : 32 * c + 5, mt0:mt1],
                                    rbh1[32 * c : 32 * c + 5,
                                         c * 512 : (c + 1) * 512],
                                    tile_position=(32 * c, 0),
                                )
                            nc.scalar.copy(eb[:, 2048 - XDVE : E], pb[:])
                            if pending is not None:
                                emit_tree(*pending)
                            pending = (eb, col)
                    emit_tree(*pending)

            # ---- phase 3: combine + sqrt + reductions ----
            if DRAIN == "cdve":
                nc.vector.tensor_reduce(
                    mindc[:],
                    mind[:].rearrange("p (a two) -> p a two", two=2),
                    axis=mybir.AxisListType.X,
                    op=AL.min,
                )
            else:
                nc.vector.tensor_tensor(
                    mindc[:], mind[:, 0:64], mind[:, 64:128], AL.min
                )
            nc.vector.tensor_scalar_max(dist[:], mindc[:], 0.0)
            nc.scalar.sqrt(mindc[:], dist[:])
            nc.vector.tensor_reduce(
                colsum[:], mindc[:], axis=mybir.AxisListType.X, op=AL.add
            )
            # partition sum via a tiny fp32 ones-matmul on the (idle) PE --
            # avoids the slow gpsimd partition_all_reduce + drain in the tail
            ones = constp.tile([128, 1], f32)
            total = constp.tile([1, 1], f32)
            nc.vector.memset(ones[:], 1.0)
            with tc.tile_pool(name="psf", bufs=1, space="PSUM") as psf:
                pt1 = psf.tile([1, 1], f32)
                nc.tensor.matmul(pt1[:], colsum[:], ones[:])
                nc.scalar.copy(total[:], pt1[:])
            nc.sync.dma_start(out_d[:], total[:])

    nc.compile()
    return nc


def _host_inputs(pred_R, pred_t, gt_R, gt_t, model_points):
    # degree-2 polynomial feature map of the shared model points
    # (layout/feature prep of the constant cloud; all batch-dependent math
    # — transforms, distances, mins — runs on device)
    p = np.asarray(model_points, np.float32).T.astype(np.float32)  # [3, M]
    x, y, z = p[0], p[1], p[2]
    feat = np.stack(
        [x, y, z, np.ones_like(x), x * x, y * y, z * z, x * y, y * z, z * x]
    ).astype(np.float32)  # [10, M]
    in_maps = []
    for c in range(NCORES):
        wmat = np.zeros((10, 128), np.float32)
        for i in range(BPC):
            b = c * BPC + i
            wl = _w_pred(np.float64(pred_R[b]), np.float64(pred_t[b])).T
            wr = _w_gt(np.float64(gt_R[b]), np.float64(gt_t[b])).T
            wmat[:, 32 * (2 * i) : 32 * (2 * i) + 5] = wl
            wmat[:, 32 * (2 * i + 1) : 32 * (2 * i + 1) + 5] = wr
        in_maps.append({"feat": feat, "wmat": wmat})
    return in_maps


LAST_RESULT = None


def kernel(pred_R, pred_t, gt_R, gt_t, model_points):
    global LAST_RESULT
    from concourse.bass_utils import run_bass_kernel_spmd

    nc = _build_graph()
    in_maps = _host_inputs(pred_R, pred_t, gt_R, gt_t, model_points)
    res = run_bass_kernel_spmd(nc, in_maps, list(range(NCORES)))
    LAST_RESULT = res
    total = sum(float(res.results[c]["out"][0, 0]) for c in range(NCORES))
    return np.float32(total / (B * M))


# revision 12
# speedup vs baseline: 1.0002x; 1.0002x over previous
"""Trainium2 Bass kernel for nn_ADDSLoss (retrieval_knn).

Math:
  pred = points @ R_p^T + t_p ; gt = points @ R_g^T + t_g          (per batch)
  out  = mean_{b,m} min_n ||pred[b,m] - gt[b,n]||

Device formulation (per batch):
  d2[m,n] = pn[m] + gn[n] - 2*pred[m].gt[n] = sum_k L[k,m]*R[k,n]   (K=5)
  with L = W_L @ F, R = W_R @ F, where F is the degree-2 polynomial
  feature map of the raw model points, built on host:
    F rows = [x, y, z, 1, x^2, y^2, z^2, x*y, y*z, z*x]   [10, M]
  and W_L/W_R are tiny per-batch 5x10 matrices (host, O(B) work).
  min_n sqrt(max(d2,0)) == sqrt(max(min_n d2, 0)), so sqrt happens on
  M values per batch, not M^2.

Sharding: data-parallel over B across the 8 cores (2 batches/core). Each
core returns its partial sum of min-distances; the final mean is the sum
of the 8 partials / (B*M) on host (a trivial 8-float reduction).

Device pipeline per core:
  phase 0: DMA F (host-built features) + stacked W matrix.
  phase 1: transform matmuls (fp32r, K=10, all four 5x10 W's stacked in
           one [10,128] lhsT) -> PSUM -> cast-evac to fp16 -> DMA the
           four 5-row side blocks to partition offsets {0,32,64,96}.
  phase 2 (DRAIN="cdve"): per (m-tile, batch, psum-tile): 4 fp16 K=5
           matmuls row-packed via tile_position -> [128,2048] d2 in
           PSUM; ACT copies cols [1024:2048] to SBUF fp16; one custom
           DVE ucode op (MIN_MIN_REDUCE_ANT: out = min(in0,in1),
           accum_out = min(s0, min out)) mins the copied half against
           the PSUM half AND min-reduces in a single DVE pass.
  phase 2 (DRAIN="tree"): baseline-style split drain - DVE
           tensor_reduce(min) eats XDVE cols from PSUM; ACT evacuates
           the rest to fp16 SBUF where a 3-level tensor_tensor(min)
           tree runs in DVE 2x mode (software-pipelined across units).
  phase 3: relu -> sqrt -> free-axis sum -> partition all-reduce -> DMA.
"""

import functools
import os

import numpy as np

B, M = 16, 4096
NCORES = 8
BPC = B // NCORES        # batches per core
NCH = M // 512           # 8 rhs chunks of 512
NMT = M // 128           # 32 m-tiles of 128
NSIDES = 2 * BPC         # L0, R0, L1, R1
BIG = 3.0e38             # running-min init (finite to avoid inf handling)
DRAIN = os.environ.get("ADDS_DRAIN", "cdve")  # "cdve" | "tree"


def _register_custom_dve():
    """Register MIN_MIN_REDUCE_ANT: out=min(in0,in1); accum=min(s0,min out)."""
    import concourse.dve_ops as dve_ops
    from concourse.dve_spec import Spec, Src0, Src1, C0, minn, lower, _has_src1
    from concourse.dve_uop import DveOpSpec

    name = "MIN_MIN_REDUCE_ANT"
    for o in dve_ops.OPS:
        if o.name == name:
            return o

    def _ref(in0, in1, c0, c1, c2):
        b = np.minimum(in0.astype(np.float32), in1.astype(np.float32)).astype(
            np.float32
        )
        return b, np.minimum(
            c0, b.reshape(b.shape[0], -1).min(axis=-1, keepdims=True)
        )

    spec = Spec(body=minn(Src0, Src1), accum=minn, accum_init=C0, reference=_ref)
    shas = {
        ver: DveOpSpec(
            name=name, opcode=0, uops=lower(spec, ver=ver), rd1_en=_has_src1(spec)
        ).sha(ver)
        for ver in ("v3", "v4")
    }
    op = dve_ops.DveOp(name, spec, subdim=False, uops_sha=shas)
    row = dve_ops._CUSTOM_DVE_ROW_BASE + len(dve_ops.OPS)
    assert row < 0x20
    dve_ops.OPS.append(op)
    dve_ops._SUB_OPCODE_FOR_NAME[name] = row
    dve_ops.CUSTOM_DVE_SPECS[name] = spec
    return op


def _quad_row(Rm, t):
    """Feature-space weights of ||R p + t||^2 over
    [x, y, z, 1, x2, y2, z2, xy, yz, zx]."""
    A = Rm.T @ Rm
    row = np.zeros(10, np.float64)
    row[0:3] = 2.0 * (Rm.T @ t)
    row[3] = float(t @ t)
    row[4:7] = np.diag(A)
    row[7] = 2.0 * A[0, 1]
    row[8] = 2.0 * A[1, 2]
    row[9] = 2.0 * A[0, 2]
    return row


def _w_pred(Rp, tp):
    W = np.zeros((5, 10), np.float64)
    W[0:3, 0:3] = Rp
    W[0:3, 3] = tp
    W[3, 3] = 1.0
    W[4] = _quad_row(Rp, tp)
    return W


def _w_gt(Rg, tg):
    W = np.zeros((5, 10), np.float64)
    W[0:3, 0:3] = -2.0 * Rg
    W[0:3, 3] = -2.0 * tg
    W[3] = _quad_row(Rg, tg)
    W[4, 3] = 1.0
    return W


@functools.lru_cache(maxsize=1)
def _build_graph():
    import concourse.mybir as mybir
    import concourse.tile as tile
    from concourse import bacc

    f32 = mybir.dt.float32
    f16 = mybir.dt.float16  # 16-bit operand dtype for the main matmuls
    AL = mybir.AluOpType

    myop = _register_custom_dve() if DRAIN == "cdve" else None

    nc = bacc.Bacc(
        "TRN2", target_bir_lowering=False, debug=False, num_devices=NCORES
    )
    f32r = mybir.dt.float32r
    feat_d = nc.declare_dram_parameter("feat", [10, M], f32r, isOutput=False)
    wm_d = nc.declare_dram_parameter("wmat", [10, 128], f32r, isOutput=False)
    out_d = nc.declare_dram_parameter("out", [1, 1], f32, isOutput=True)

    with tile.TileContext(nc) as tc:
        with (
            tc.tile_pool(name="constp", bufs=1) as constp,
        ):
            F = constp.tile([10, M], f32r)
            wm = constp.tile([10, 128], f32r)
            sbig = constp.tile([128, M], f16)
            mind = constp.tile([128, 2 * NMT * BPC], f32)
            mindc = constp.tile([128, NMT * BPC], f32)
            dist = constp.tile([128, NMT * BPC], f32)
            colsum = constp.tile([128, 1], f32)
            dummy = constp.tile([128, 1], f32)
            # per-side operand tensors replicated at partition offsets
            # {0,32,64,96} so four K=5 matmuls can run concurrently in the
            # PE array via tile_position row-packing; split into halves so
            # phase 2 can start as soon as the first half is transformed
            lrh = [
                [
                    constp.tile([128, M // 2], f16, name=f"lr{s}h{h}")
                    for h in range(2)
                ]
                for s in range(NSIDES)
            ]

            # ---- phase 0: inputs ----
            nc.sync.dma_start(wm[:], wm_d[:])
            # touch Sqrt now so its ACT table set loads during the prelude
            # (Copy lives in every set, so no second table switch later);
            # scale=0/bias=1 keeps the probe input in sqrt's valid range
            nc.scalar.activation(
                dummy[0:10, 0:1],
                wm[:, 0:1],
                mybir.ActivationFunctionType.Sqrt,
                bias=1.0,
                scale=0.0,
            )
            # chunked so the first transform matmul starts after 1/8 of F;
            # issue alternately on two queues so the half-0 chunks land fast
            for q in range(NCH):
                eng = nc.sync if q % 2 == 0 else nc.scalar
                eng.dma_start(
                    F[:, q * 512 : (q + 1) * 512],
                    feat_d[:, q * 512 : (q + 1) * 512],
                )

            # ---- phase 1: L/R transforms (fp32 exact) ----
            # All four 5x10 transform matrices are stacked into one
            # [10, 128] lhsT (side s at columns 32s..32s+5): the fp32
            # matmul cost is per-row, independent of M, so one stacked
            # matmul per chunk does the work of four.  The [128, 512]
            # outputs are cast-evacuated to fp16 and the four 5-row
            # blocks DMA'd to partition 0 where the main matmuls expect
            # their K operands.
            with tc.tile_pool(name="pst", bufs=4, space="PSUM") as pst:
                for cc in range(NCH):
                    h, hc = cc // (NCH // 2), cc % (NCH // 2)
                    tp = pst.tile([128, 512], f32, tag="tp")
                    nc.tensor.matmul(tp[:], wm[:], F[:, cc * 512 : (cc + 1) * 512])
                    csl = slice(cc * 512, (cc + 1) * 512)
                    if cc % 2 == 0:
                        nc.scalar.copy(sbig[:, csl], tp[:])
                    else:
                        nc.vector.tensor_copy(sbig[:, csl], tp[:])
                    if hc == NCH // 2 - 1:
                        # replicate this half's four 5-row side blocks to the
                        # four 32-row offsets.  DMA *issue* costs ~800ns per
                        # descriptor on a sequencer, so spread the 16 DMAs
                        # over four otherwise-idle sequencers
                        hof = h * (M // 2)
                        # keep ACT's queue clear of DMA issue cost: its
                        # sequencer must start phase-2 copies immediately
                        emitters = [nc.sync, nc.gpsimd, nc.sync, nc.gpsimd]
                        for s in range(NSIDES):
                            for i in range(4):
                                emitters[i].dma_start(
                                    lrh[s][h][32 * i : 32 * i + 5, :],
                                    sbig[32 * s : 32 * s + 5,
                                         hof : hof + M // 2],
                                )

            # ---- phase 2: pairwise d2 + min-reduce ----
            if DRAIN == "cdve":
                # Per [128,2048] PSUM tile: ACT copies the top half to fp16
                # SBUF; one custom DVE op computes min(copied, psum-bottom)
                # elementwise AND min-reduces it to mind[:, col] in a single
                # pass (2 d2 columns consumed per DVE cycle-column).
                # Quarter-tile schedule: four [128,1024] PSUM slots per
                # iteration.  ACT copies quarters q1/q3 to SBUF while the
                # custom ops pair them with q0/q2 straight from PSUM, so
                # the copy is off the DVE critical path and the copy-slots
                # free early for the next iteration's matmuls.
                # (half, b, mt) order: all half-0 units run before any
                # half-1 unit, so phase 2 starts as soon as the half-0
                # operand replicas land (half-1 transforms overlap it).
                # Per unit: copy-quarter matmuls emitted first (its PSUM
                # slot frees early via the ACT copy), custom-quarter
                # second; row-groups alternate between units so adjacent
                # quarters overlap in the PE array.
                with tc.tile_pool(name="psm", bufs=4, space="PSUM") as psm, \
                     tc.tile_pool(name="cbp", bufs=3) as cbp, \
                     tc.tile_pool(name="dmp", bufs=2) as dmp:
                    u = 0
                    for half in range(2):
                        for b in range(BPC):
                            for mt in range(NMT):
                                lb = lrh[2 * b][mt // (NMT // 2)]
                                rbh = lrh[2 * b + 1][half]
                                mt0 = (mt % (NMT // 2)) * 128
                                mt1 = mt0 + 128
                                pq = [None, None]
                                for qi in range(2):  # 0 = copy-q, 1 = custom-q
                                    c0 = (1 - qi) * 1024
                                    pt = psm.tile([128, 1024], f32, tag="ps")
                                    for c in range(2):
                                        pos = 64 * ((u + qi) % 2) + 32 * c
                                        nc.tensor.matmul(
                                            pt[:, c * 512 : (c + 1) * 512],
                                            lb[pos : pos + 5, mt0:mt1],
                                            rbh[pos : pos + 5,
                                                c0 + c * 512 : c0 + (c + 1) * 512],
                                            tile_position=(pos, 0),
                                        )
                                    pq[qi] = pt
                                cb = cbp.tile([128, 1024], f32, tag="cb")
                                dmpt = dmp.tile([128, 1024], f32, tag="dm")
                                nc.scalar.copy(cb[:], pq[0][:])
                                col = 2 * (mt * BPC + b) + half
                                nc.vector._custom_dve(
                                    myop,
                                    out=dmpt[:],
                                    in0=cb[:],
                                    in1=pq[1][:],
                                    s0=BIG,
                                    s1=0.0,
                                    accum_out=mind[:, col : col + 1],
                                )
                                u += 1
            else:
                # baseline-style split drain, rebalanced: DVE
                # tensor_reduce(min) eats XDVE cols straight from PSUM;
                # ACT evacuates the rest to fp16 SBUF for a 3-level DVE
                # 2x tensor_tensor(min) tree (pipelined across units).
                XDVE = 1000
                E = 4096 - XDVE
                minda = mind  # reuse: cols 0..63 direct, 64..127 tree

                def emit_tree(eb, col):
                    l1 = ebp.tile([128, E // 2], f16, tag="l1")
                    l2 = ebp.tile([128, E // 4], f16, tag="l2")
                    l3 = ebp.tile([128, E // 8], f16, tag="l3")
                    nc.vector.tensor_tensor(
                        l1[:], eb[:, 0 : E // 2], eb[:, E // 2 : E], AL.min
                    )
                    nc.vector.tensor_tensor(
                        l2[:], l1[:, 0 : E // 4], l1[:, E // 4 : E // 2], AL.min
                    )
                    nc.vector.tensor_tensor(
                        l3[:], l2[:, 0 : E // 8], l2[:, E // 8 : E // 4], AL.min
                    )
                    nc.vector.tensor_reduce(
                        minda[:, 64 + col : 65 + col],
                        l3[:],
                        axis=mybir.AxisListType.X,
                        op=AL.min,
                    )

                with tc.tile_pool(name="psm", bufs=2, space="PSUM") as psm, \
                     tc.tile_pool(name="ebp", bufs=4) as ebp:
                    pending = None
                    for b in range(BPC):
                        for mt in range(NMT):
                            lb = lrh[2 * b][mt // (NMT // 2)]
                            rbh0 = lrh[2 * b + 1][0]
                            rbh1 = lrh[2 * b + 1][1]
                            mt0 = (mt % (NMT // 2)) * 128
                            mt1 = mt0 + 128
                            col = mt * BPC + b
                            eb = ebp.tile([128, E], f16, tag="eb")
                            pa = psm.tile([128, 2048], f32, tag="ps")
                            for c in range(4):
                                nc.tensor.matmul(
                                    pa[:, c * 512 : (c + 1) * 512],
                                    lb[32 * c : 32 * c + 5, mt0:mt1],
                                    rbh0[32 * c : 32 * c + 5,
                                         c * 512 : (c + 1) * 512],
                                    tile_position=(32 * c, 0),
                                )
                            nc.vector.tensor_reduce(
                                mind[:, col : col + 1],
                                pa[:, 0:XDVE],
                                axis=mybir.AxisListType.X,
                                op=AL.min,
                            )
                            nc.scalar.copy(eb[:, 0 : 2048 - XDVE], pa[:, XDVE:2048])
                            pb = psm.tile([128, 2048], f32, tag="ps")
                            for c in range(4):
                                nc.tensor.matmul(
                                    pb[:, c * 512 : (c + 1) * 512],
                                    lb[32 * c : 32 * c + 5, mt0:mt1],
                                    rbh1[32 * c : 32 * c + 5,
                                         c * 512 : (c + 1) * 512],
                                    tile_position=(32 * c, 0),
                                )
                            nc.scalar.copy(eb[:, 2048 - XDVE : E], pb[:])
                            if pending is not None:
                                emit_tree(*pending)
                            pending = (eb, col)
                    emit_tree(*pending)

            # ---- phase 3: combine + sqrt + reductions ----
            if DRAIN == "cdve":
                nc.vector.tensor_reduce(
                    mindc[:],
                    mind[:].rearrange("p (a two) -> p a two", two=2),
                    axis=mybir.AxisListType.X,
                    op=AL.min,
                )
            else:
                nc.vector.tensor_tensor(
                    mindc[:], mind[:, 0:64], mind[:, 64:128], AL.min
                )
            nc.vector.tensor_scalar_max(dist[:], mindc[:], 0.0)
            nc.scalar.sqrt(mindc[:], dist[:])
            nc.vector.tensor_reduce(
                colsum[:], mindc[:], axis=mybir.AxisListType.X, op=AL.add
            )
            # partition sum via a tiny fp32 ones-matmul on the (idle) PE --
            # avoids the slow gpsimd partition_all_reduce + drain in the tail
            ones = constp.tile([128, 1], f32)
            total = constp.tile([1, 1], f32)
            nc.vector.memset(ones[:], 1.0)
            with tc.tile_pool(name="psf", bufs=1, space="PSUM") as psf:
                pt1 = psf.tile([1, 1], f32)
                nc.tensor.matmul(pt1[:], colsum[:], ones[:])
                nc.scalar.copy(total[:], pt1[:])
            nc.sync.dma_start(out_d[:], total[:])

    nc.compile()
    return nc


def _host_inputs(pred_R, pred_t, gt_R, gt_t, model_points):
    # degree-2 polynomial feature map of the shared model points
    # (layout/feature prep of the constant cloud; all batch-dependent math
    # — transforms, distances, mins — runs on device)
    p = np.asarray(model_points, np.float32).T.astype(np.float32)  # [3, M]
    x, y, z = p[0], p[1], p[2]
    feat = np.stack(
        [x, y, z, np.ones_like(x), x * x, y * y, z * z, x * y, y * z, z * x]
    ).astype(np.float32)  # [10, M]
    in_maps = []
    for c in range(NCORES):
        wmat = np.zeros((10, 128), np.float32)
        for i in range(BPC):
            b = c * BPC + i
            wl = _w_pred(np.float64(pred_R[b]), np.float64(pred_t[b])).T
            wr = _w_gt(np.float64(gt_R[b]), np.float64(gt_t[b])).T
            wmat[:, 32 * (2 * i) : 32 * (2 * i) + 5] = wl
            wmat[:, 32 * (2 * i + 1) : 32 * (2 * i + 1) + 5] = wr
        in_maps.append({"feat": feat, "wmat": wmat})
    return in_maps


LAST_RESULT = None


def kernel(pred_R, pred_t, gt_R, gt_t, model_points):
    global LAST_RESULT
    from concourse.bass_utils import run_bass_kernel_spmd

    nc = _build_graph()
    in_maps = _host_inputs(pred_R, pred_t, gt_R, gt_t, model_points)
    res = run_bass_kernel_spmd(nc, in_maps, list(range(NCORES)))
    LAST_RESULT = res
    total = sum(float(res.results[c]["out"][0, 0]) for c in range(NCORES))
    return np.float32(total / (B * M))
